# revision 19
# baseline (speedup 1.0000x reference)
"""CenterLoss on Trainium2 (8 NeuronCores, raw Bass).

reference: mean_i ||x_i - centers[labels_i]||_2  over batch of 4096, feat 512.

Strategy (per the class-parallel/data-parallel hint): centers is 100000x512 but
only the 4096 gathered rows matter. The gather centers[labels] is done on host
(tiny: 4096x512 = 8MB), then the batch is sharded data-parallel across the 8
cores (512 rows each). Each core computes its 512 Euclidean distances
on-device: DVE subtract, ACT square with fused row-sum accumulation (f32
accumulator), ACT sqrt. Host sums the 8x512 distances and divides by 4096.

Perf notes:
- x and the gathered centers are packed side-by-side per row ([512, 1024]) and
  shipped as bf16 (1MB/core): halves the DMA and doubles DVE throughput while
  the f32 accumulator keeps end-to-end relative error ~1e-5.
- The load is split into 4 chunks (one per 128-row group) so the DVE subtract
  and ACT square of group t overlap group t+1's DMA. One semaphore per chunk:
  DMA completion order across queues is not FIFO.
- Every instruction carries at most ONE semaphore wait (this walrus build
  rejects more), which is why raw Bass is used instead of Tile (Tile's
  kernel-tail drain needs multi-sem waits).
- A dummy Square at ACT program start pulls the ~1.3us activation-table load
  under the DMA window.
- The ACT accumulator flush is not interlocked with a later ACT instruction's
  read, so the final sqrt is gated on the four accumulate semaphores.
- The jitted shard_map runner is built once and cached: rebuilding it per call
  (as run_bass_kernel_spmd does) costs ~0.4s of retracing per invocation.
"""

import numpy as np
import ml_dtypes

import concourse.bass as bass
import concourse.mybir as mybir

N_CORES = 8
BATCH = 4096
FEAT = 512
ROWS = BATCH // N_CORES  # 512 rows per core
P = 128                  # SBUF partitions
T = ROWS // P            # 4 row-groups of 128 per core

_NC_CACHE = None
_RUNNER = None
LAST_RESULTS = None  # test harness introspection (exec_time_ns when tracing)


def _build_nc():
    f32 = mybir.dt.float32
    bf16 = mybir.dt.bfloat16
    nc = bass.Bass(enable_partition_id=False)
    xc = nc.dram_tensor("xc", [ROWS, 2 * FEAT], bf16, kind="ExternalInput")
    dist_out = nc.dram_tensor("dist", [P, T], f32, kind="ExternalOutput")

    # partition p holds rows {t*128+p : t in 0..T}: [128, 4, 1024]
    xc_v = xc.rearrange("(t p) f -> p t f", p=P)

    with (
        nc.sbuf_tensor("xct", [P, T, 2 * FEAT], bf16) as xct,
        nc.sbuf_tensor("d", [P, T, FEAT], bf16) as d,
        nc.sbuf_tensor("sq", [P, T, FEAT], bf16) as sq,
        nc.sbuf_tensor("warm", [P, 1], f32) as warm,
        nc.sbuf_tensor("ssum", [P, T], f32) as ssum,
        nc.sbuf_tensor("dist_sb", [P, T], f32) as dist_sb,
        nc.semaphore("s_in0") as s_in0,
        nc.semaphore("s_in1") as s_in1,
        nc.semaphore("s_in2") as s_in2,
        nc.semaphore("s_in3") as s_in3,
        nc.semaphore("s_sub") as s_sub,
        nc.semaphore("s_acc") as s_acc,
        nc.Block() as block,
    ):
        s_in = [s_in0, s_in1, s_in2, s_in3]

        @block.sync
        def _(sync: bass.BassEngine):
            # chunked load: group t's compute overlaps group t+1's DMA
            for t in range(T):
                sync.dma_start(out=xct[:, t, :], in_=xc_v[:, t, :]).then_inc(
                    s_in[t], 16
                )
            sync.wait_ge(s_acc, T + 1)
            sync.dma_start(
                out=dist_out[:], in_=dist_sb[:], single_packet=True
            ).then_inc(s_sub, 16)
            sync.wait_ge(s_sub, T + 16)

        @block.vector
        def _(vector: bass.BassEngine):
            for t in range(T):
                vector.wait_ge(s_in[t], 16)
                vector.tensor_sub(
                    d[:, t, :], xct[:, t, :FEAT], xct[:, t, FEAT:]
                ).then_inc(s_sub, 1)

        @block.scalar
        def _(scalar: bass.BassEngine):
            # warm the activation table while the input DMA is in flight
            one = nc.const_aps.tensor(1.0, (P, 1), mybir.dt.float32)
            scalar.activation(warm[:], one, mybir.ActivationFunctionType.Square)
            for t in range(T):
                scalar.wait_ge(s_sub, t + 1)
                scalar.activation(
                    sq[:, t, :],
                    d[:, t, :],
                    mybir.ActivationFunctionType.Square,
                    accum_out=ssum[:, t : t + 1],
                ).then_inc(s_acc, 1)
            # The accumulator flush is NOT interlocked with a following ACT
            # instruction's read — gate the sqrt on all four accum sems.
            scalar.wait_ge(s_acc, T)
            scalar.sqrt(dist_sb[:], ssum[:]).then_inc(s_acc, 1)

    return nc


def _get_nc():
    global _NC_CACHE
    if _NC_CACHE is None:
        _NC_CACHE = _build_nc()
    return _NC_CACHE


def _get_runner():
    """Build the jitted shard_map runner once; jax.jit caches by function
    identity, so rebuilding per call would re-trace every time."""
    global _RUNNER
    if _RUNNER is None:
        import jax
        from jax.experimental.shard_map import shard_map
        from jax.sharding import Mesh, PartitionSpec
        from concourse.bass2jax import _bass_exec_p, install_neuronx_cc_hook

        install_neuronx_cc_hook()
        nc = _get_nc()
        out_avals = (jax.core.ShapedArray((P, T), np.float32),)

        def _body(xc_arr, zero_out):
            outs = _bass_exec_p.bind(
                xc_arr,
                zero_out,
                out_avals=out_avals,
                in_names=("xc", "dist"),
                out_names=("dist",),
                lowering_input_output_aliases=(),
                sim_require_finite=True,
                sim_require_nnan=True,
                nc=nc,
            )
            return tuple(outs)

        devices = jax.devices()[:N_CORES]
        assert len(devices) == N_CORES
        mesh = Mesh(np.asarray(devices), ("core",))
        _RUNNER = jax.jit(
            shard_map(
                _body,
                mesh=mesh,
                in_specs=(PartitionSpec("core"), PartitionSpec("core")),
                out_specs=(PartitionSpec("core"),),
                check_rep=False,
            ),
            donate_argnums=(1,),
            keep_unused=True,
        )
    return _RUNNER


def kernel(x, labels, centers, _trace=False):
    global LAST_RESULTS
    x = np.asarray(x, dtype=np.float32)
    labels = np.asarray(labels).astype(np.int64)
    centers = np.asarray(centers, dtype=np.float32)

    own = centers[labels]  # [BATCH, FEAT] host gather
    xc = np.concatenate([x, own], axis=1).astype(ml_dtypes.bfloat16)

    if _trace:
        # profiling path: run_bass_kernel_spmd captures NTFF + exec_time_ns
        from concourse.bass_utils import run_bass_kernel_spmd

        in_maps = [
            {"xc": xc[k * ROWS : (k + 1) * ROWS]} for k in range(N_CORES)
        ]
        res = run_bass_kernel_spmd(
            _get_nc(), in_maps, list(range(N_CORES)), trace=True
        )
        LAST_RESULTS = res
        total = 0.0
        for r in res.results:
            total += float(np.asarray(r["dist"], dtype=np.float64).sum())
        return np.float32(total / BATCH)

    run = _get_runner()
    # device c gets rows [512c, 512c+512) — exactly the per-core shard
    (dist,) = run(xc, np.zeros((N_CORES * P, T), np.float32))
    total = float(np.asarray(dist, dtype=np.float64).sum())
    return np.float32(total / BATCH)
